# revision 17
# baseline (speedup 1.0000x reference)
"""SSD-style multibox loss (Huber loc + softmax conf with hard-negative
mining) on 8 Trainium2 NeuronCores, pure data-parallel over the batch.

Full inputs [32, 8732, ...] come in; each core processes 4 batch rows.
Per core the 4x8732 anchors are laid out flat across 128 partitions as
4 row-blocks of 32 partitions x 273 anchor-"groups" (32*273 = 8736, the
last 4 slots of each row-block's final partition are padding).

Device computes, per core:
  - sumexp / lse per anchor (ACT exp + DVE reduce over the 81 classes)
  - pos mask / pos count (from actual_bbox_deltas)
  - Huber localization sum over positives
  - S1 = sum(actual_labels * pred_labels)   (per-chunk accumulators)
  - S2 = sum_pos lse, S3 = sum_pos pred0, S4 = sum_all pred0
  - hard-negative top-k sum via an 11-step vectorized binary search on the
    threshold t_r per batch row (k_r = 3*pos_count_r), using
    sum_topk = sum(v*[v>t]) + t*(k - count(v>t))
Host combines the 8 cores' scalar partials and performs the final division.

One-hot-label identity used:  conf = lse - dot(labels, pred);  for negative
anchors dot = pred[:, 0], so
  sum_pos conf = S2 - (S1 - (S4 - S3)).

DMA: pred_labels + bbox stream on the Activation HWDGE queue, actual_labels
on the Sync HWDGE queue, so the two big streams issue descriptors in
parallel. The mining binary search runs entirely on ACT+PE so the DVE
stream (sumexp reduces + label dots) is never blocked by it.
"""

import numpy as np

import concourse.bass as bass
import concourse.bacc as bacc
import concourse.tile as tile
import concourse.mybir as mybir
from concourse.bass_utils import run_bass_kernel_spmd

F32 = mybir.dt.float32
AX = mybir.AxisListType
OP = mybir.AluOpType
AF = mybir.ActivationFunctionType

B, P, C = 32, 8732, 81
NCORES = 8
BL = B // NCORES            # batch rows per core = 4
PPR = 32                    # partitions per row-block
G = 273                     # anchor groups per partition (32*273 = 8736)
NV = P - (PPR - 1) * G      # valid groups on last partition of a block = 269
NEG_BIG = -1.0e30
NITER = 11                  # binary-search iterations (range [0, 32))
T0 = 16.0
NF = 32                     # output partial columns

CHP = 91                    # pred DMA chunk (3 chunks)
NCHP = G // CHP
CHL = 39                    # labels DMA chunk (7 chunks) = compute sub-chunk
NCHL = G // CHL

# column map of the [1, NF] per-core output
COL_LOC, COL_S2, COL_S3, COL_S4, COL_POS, COL_NEG = 0, 1, 2, 3, 4, 5
COL_S1 = 8                  # .. COL_S1 + NCHL - 1


def _dma_grid(nc, eng, dst, dram, inner, g0, g1, padz):
    """Fill dst[0:128, 0:(g1-g0), :inner] from dram [BL, P, inner] where
    partition 32*r+q holds groups [q*G + g0, q*G + g1) of row r. Handles
    the ragged tail (groups >= NV invalid on the last partition of each
    block) with separate DMAs + a pad fill."""
    gv = min(g1, NV)        # groups valid on every partition
    # main body: all 128 partitions, groups [g0, gv)
    src = bass.AP(dram, g0 * inner,
                  [[P * inner, BL], [G * inner, PPR], [inner, gv - g0],
                   [1, inner]])
    eng.dma_start(dst[:, 0:gv - g0, :], src)
    if g1 > NV:
        # groups [NV, g1) valid only on partitions q < 31 of each block
        for r in range(BL):
            p0 = r * PPR
            src = bass.AP(dram, r * P * inner + NV * inner,
                          [[G * inner, PPR - 1], [inner, g1 - NV], [1, inner]])
            eng.dma_start(dst[p0:p0 + PPR - 1, NV - g0:g1 - g0, :], src)
            eng.dma_start(dst[p0 + PPR - 1:p0 + PPR, NV - g0:g1 - g0, :],
                          padz[:, 0:g1 - NV, 0:inner])


def build():
    nc = bacc.Bacc("TRN2", target_bir_lowering=False, debug=False)

    d_ab = nc.dram_tensor("actual_bbox_deltas", [BL, P, 4], F32, kind="ExternalInput")
    d_al = nc.dram_tensor("actual_labels", [BL, P, C], F32, kind="ExternalInput")
    d_pb = nc.dram_tensor("pred_bbox_deltas", [BL, P, 4], F32, kind="ExternalInput")
    d_pl = nc.dram_tensor("pred_labels", [BL, P, C], F32, kind="ExternalInput")
    d_out = nc.dram_tensor("out", [1, NF], F32, kind="ExternalOutput")

    with tile.TileContext(nc) as tc:
        with (
            tc.tile_pool(name="const", bufs=1) as constp,
            tc.tile_pool(name="resident", bufs=1) as resp,
            tc.tile_pool(name="bbox", bufs=1) as bbp,
            tc.tile_pool(name="hub", bufs=1) as hubp,
            tc.tile_pool(name="expj", bufs=2) as expp,
            tc.tile_pool(name="lblchunk", bufs=2) as lblp,
            tc.tile_pool(name="small", bufs=2) as smallp,
            tc.tile_pool(name="mine", bufs=2) as minep,
            tc.tile_pool(name="psum", bufs=2, space="PSUM") as psump,
            tc.tile_pool(name="dram", bufs=1, space="DRAM") as dramp,
        ):
            # ---- pad-fill constant buffers (engine memsets cannot target
            # partition bases that aren't 32-aligned; pads are DMA-filled
            # from small internal-DRAM buffers instead) ----
            stg = constp.tile([128, 4, C], F32, tag="stg")
            nc.gpsimd.memset(stg[:, :, :], 0.0)
            stageneg = constp.tile([128, 4], F32, tag="stageneg")
            nc.gpsimd.memset(stageneg[:, :], NEG_BIG)
            padz = dramp.tile([1, 4, C], F32, tag="padz")
            nc.sync.dma_start(padz[:, :, :], stg[0:1, :, :])
            padneg = dramp.tile([1, 4], F32, tag="padneg")
            nc.sync.dma_start(padneg[:, :], stageneg[0:1, :])

            # ---- constants ----
            blockones = constp.tile([128, 128], F32)
            nc.gpsimd.memset(blockones[:, :], 0.0)
            for r in range(BL):
                nc.gpsimd.memset(
                    blockones[r * PPR:(r + 1) * PPR, r * PPR:(r + 1) * PPR], 1.0)
            onescol = constp.tile([128, 1], F32)
            nc.gpsimd.memset(onescol[:, :], 1.0)
            fpart = constp.tile([128, NF], F32)
            nc.gpsimd.memset(fpart[:, :], 0.0)
            negone = constp.tile([128, 1], F32)
            nc.gpsimd.memset(negone[:, :], -1.0)

            # ---- bbox + pred DMAs (Activation HWDGE queue) ----
            abt = bbp.tile([128, G, 4], F32, tag="abt")
            pbt = bbp.tile([128, G, 4], F32, tag="pbt")
            _dma_grid(nc, nc.scalar, abt, d_ab, 4, 0, G, padz)
            _dma_grid(nc, nc.scalar, pbt, d_pb, 4, 0, G, padz)

            pred = resp.tile([128, G, C], F32, tag="pred")
            for k in range(NCHP):
                _dma_grid(nc, nc.scalar, pred[:, k * CHP:(k + 1) * CHP, :],
                          d_pl, C, k * CHP, (k + 1) * CHP, padz)

            # ---- bbox compute ----
            absmax = bbp.tile([128, G], F32, tag="absmax")
            nc.vector.tensor_reduce(absmax[:, :], abt[:, :, :], AX.X, OP.max,
                                    apply_absolute_value=True)
            posmask = bbp.tile([128, G], F32, tag="posmask")
            nc.vector.tensor_scalar(posmask[:, :], absmax[:, :], 0.0, None, OP.is_gt)

            pospart = bbp.tile([128, 1], F32, tag="pospart")
            nc.vector.tensor_reduce(pospart[:, :], posmask[:, :], AX.X, OP.add)
            nc.vector.tensor_copy(fpart[:, COL_POS:COL_POS + 1], pospart[:, :])
            pos_rep = psump.tile([128, 1], F32, tag="posrep")
            nc.tensor.matmul(pos_rep[:, :], blockones[:, :], pospart[:, :])
            # kcol = 3*pos ; sign-count threshold negk2 = n_tot - 6*pos
            kcol = bbp.tile([128, 1], F32, tag="kcol")
            nc.vector.tensor_scalar(kcol[:, :], pos_rep[:, :], 3.0, None, OP.mult)
            negk2 = bbp.tile([128, 1], F32, tag="negk2")
            nc.vector.tensor_scalar(negk2[:, :], pos_rep[:, :], -6.0,
                                    float(PPR * G), OP.mult, OP.add)

            # Huber loc loss (3 scratch tiles, in-place chains)
            dt_ = hubp.tile([128, G, 4], F32, tag="hd")
            nc.vector.tensor_sub(dt_[:, :, :], pbt[:, :, :], abt[:, :, :])
            nc.scalar.activation(dt_[:, :, :], dt_[:, :, :], AF.Abs)  # a = |d|
            mt = hubp.tile([128, G, 4], F32, tag="hm")
            nc.vector.tensor_single_scalar(mt[:, :, :], dt_[:, :, :], 1.0, OP.min)
            st = hubp.tile([128, G, 4], F32, tag="hs")
            nc.scalar.activation(st[:, :, :], mt[:, :, :], AF.Square,
                                 scale=float(np.sqrt(0.5)))       # 0.5*m^2
            nc.scalar.activation(dt_[:, :, :], dt_[:, :, :], AF.Relu,
                                 bias=negone[:, :])               # relu(a-1)
            nc.vector.tensor_add(st[:, :, :], st[:, :, :], dt_[:, :, :])
            hpb = hubp.tile([128, G], F32, tag="hpb")
            nc.vector.tensor_reduce(hpb[:, :], st[:, :, :], AX.X, OP.add)
            hjunk = hubp.tile([128, G], F32, tag="hjunk")
            nc.vector.scalar_tensor_tensor(
                hjunk[:, :], hpb[:, :], 0.25, posmask[:, :], OP.mult, OP.mult,
                accum_out=fpart[:, COL_LOC:COL_LOC + 1])

            # ---- exp + per-anchor sumexp over pred (sub-chunks of CHL) ----
            sumexp = resp.tile([128, G], F32, tag="sumexp")
            for k in range(NCHL):
                sl = pred[:, k * CHL:(k + 1) * CHL, :]
                ex = expp.tile([128, CHL, C], F32, tag="exp")
                nc.scalar.activation(ex[:, :, :], sl, AF.Exp)
                nc.vector.tensor_reduce(sumexp[:, k * CHL:(k + 1) * CHL],
                                        ex[:, :, :], AX.X, OP.add)

            lse = resp.tile([128, G], F32, tag="lse")
            nc.scalar.activation(lse[:, :], sumexp[:, :], AF.Ln)
            pred0 = pred[:, :, 0]
            nconf = resp.tile([128, G], F32, tag="nconf")
            nc.vector.tensor_sub(nconf[:, :], lse[:, :], pred0)
            masked = resp.tile([128, G], F32, tag="masked")
            nc.vector.scalar_tensor_tensor(
                masked[:, :], posmask[:, :], NEG_BIG, nconf[:, :], OP.mult, OP.add)
            for r in range(BL):
                nc.sync.dma_start(
                    masked[r * PPR + PPR - 1:r * PPR + PPR, NV:G], padneg[:, :])

            # S2, S3, S4
            j2 = smallp.tile([128, G], F32, tag="sjunk")
            nc.vector.scalar_tensor_tensor(
                j2[:, :], posmask[:, :], 0.0, lse[:, :], OP.bypass, OP.mult,
                accum_out=fpart[:, COL_S2:COL_S2 + 1])
            j3 = smallp.tile([128, G], F32, tag="sjunk")
            nc.vector.scalar_tensor_tensor(
                j3[:, :], posmask[:, :], 0.0, pred0, OP.bypass, OP.mult,
                accum_out=fpart[:, COL_S3:COL_S3 + 1])
            nc.vector.tensor_reduce(fpart[:, COL_S4:COL_S4 + 1], pred0, AX.X, OP.add)

            # ---- hard-negative mining: binary search on t per row (ACT+PE
            # only, so the DVE stream of label dots is never blocked) ----
            negt = minep.tile([128, 1], F32, tag="negt")
            nc.gpsimd.memset(negt[:, :], -T0)
            for i in range(NITER):
                cjunk = minep.tile([128, G], F32, tag="cjunk")
                cnt = minep.tile([128, 1], F32, tag="cnt")
                # sum(sign(masked - t)) = cnt_gt - cnt_le   (per partition)
                nc.scalar.activation(cjunk[:, :], masked[:, :], AF.Sign,
                                     bias=negt[:, :], accum_out=cnt[:, :])
                srep = psump.tile([128, 1], F32, tag="srep")
                nc.tensor.matmul(srep[:, :], blockones[:, :], cnt[:, :])
                # s = sign(sum_rep - (2k - n)) : +1 -> count>k -> t too low
                sdir = minep.tile([128, 1], F32, tag="sdir")
                nc.scalar.activation(sdir[:, :], srep[:, :], AF.Sign,
                                     bias=negk2[:, :])
                delta = T0 / (2 ** (i + 1))
                negt2 = minep.tile([128, 1], F32, tag="negt")
                nc.scalar.activation(negt2[:, :], sdir[:, :], AF.Identity,
                                     bias=negt[:, :], scale=-delta)
                negt = negt2

            # ---- actual_labels stream (Sync HWDGE queue) + dot products ----
            for k in range(NCHL):
                lbl = lblp.tile([128, CHL, C], F32, tag="lbl")
                _dma_grid(nc, nc.sync, lbl, d_al, C, k * CHL, (k + 1) * CHL, padz)
                dj = lblp.tile([128, CHL, C], F32, tag="dotjunk")
                nc.vector.scalar_tensor_tensor(
                    dj[:, :, :], lbl[:, :, :], 0.0,
                    pred[:, k * CHL:(k + 1) * CHL, :], OP.bypass, OP.mult,
                    accum_out=fpart[:, COL_S1 + k:COL_S1 + k + 1])

            # final mining pass (emitted after the label dots so these
            # mining-gated ops sit at the tail of the in-order DVE stream)
            tcol = minep.tile([128, 1], F32, tag="tcol")
            nc.vector.tensor_scalar(tcol[:, :], negt[:, :], -1.0, None, OP.mult)
            fjunk = minep.tile([128, G], F32, tag="fjunk")
            cntf = minep.tile([128, 1], F32, tag="cntf")
            nc.vector.tensor_scalar(fjunk[:, :], masked[:, :], tcol[:, :], 0.0,
                                    OP.is_gt, OP.add, accum_out=cntf[:, :])
            fjunk2 = minep.tile([128, G], F32, tag="fjunk")
            negsump = minep.tile([128, 1], F32, tag="negsump")
            nc.vector.scalar_tensor_tensor(
                fjunk2[:, :], masked[:, :], tcol[:, :], masked[:, :],
                OP.is_gt, OP.mult, accum_out=negsump[:, :])
            # contrib = negsum - t*cntf + t*kcol/PPR
            c1 = minep.tile([128, 1], F32, tag="c1")
            nc.vector.tensor_mul(c1[:, :], tcol[:, :], cntf[:, :])
            d1 = minep.tile([128, 1], F32, tag="d1")
            nc.vector.scalar_tensor_tensor(
                d1[:, :], kcol[:, :], 1.0 / PPR, tcol[:, :], OP.mult, OP.mult)
            e1 = minep.tile([128, 1], F32, tag="e1")
            nc.vector.tensor_sub(e1[:, :], negsump[:, :], c1[:, :])
            nc.vector.tensor_add(fpart[:, COL_NEG:COL_NEG + 1], e1[:, :], d1[:, :])

            # ---- final cross-partition reduce and output ----
            opsum = psump.tile([1, NF], F32, tag="opsum")
            nc.tensor.matmul(opsum[:, :], onescol[:, :], fpart[:, :])
            osb = constp.tile([1, NF], F32)
            nc.vector.tensor_copy(osb[:, :], opsum[:, :])
            nc.sync.dma_start(d_out[:, :], osb[:, :])

    nc.compile()
    return nc


_nc = None


def kernel(actual_bbox_deltas, actual_labels, pred_bbox_deltas, pred_labels):
    global _nc
    if _nc is None:
        _nc = build()

    in_maps = []
    for core in range(NCORES):
        r0 = core * BL
        in_maps.append({
            "actual_bbox_deltas": np.ascontiguousarray(
                actual_bbox_deltas[r0:r0 + BL], np.float32),
            "actual_labels": np.ascontiguousarray(
                actual_labels[r0:r0 + BL], np.float32),
            "pred_bbox_deltas": np.ascontiguousarray(
                pred_bbox_deltas[r0:r0 + BL], np.float32),
            "pred_labels": np.ascontiguousarray(
                pred_labels[r0:r0 + BL], np.float32),
        })

    res = run_bass_kernel_spmd(_nc, in_maps, core_ids=list(range(NCORES)))
    loc = conf = pos = 0.0
    for core in range(NCORES):
        o = res.results[core]["out"][0].astype(np.float64)
        s1 = o[COL_S1:COL_S1 + NCHL].sum()
        loc += o[COL_LOC]
        conf += o[COL_S2] - s1 + o[COL_S4] - o[COL_S3] + o[COL_NEG]
        pos += o[COL_POS]
    if pos == 0:
        return (np.float32(0.0), np.float32(0.0))
    return (np.float32(loc / pos), np.float32(conf / pos))


# revision 18
# speedup vs baseline: 1.2110x; 1.2110x over previous
"""SSD-style multibox loss (Huber loc + softmax conf with hard-negative
mining) on 8 Trainium2 NeuronCores, pure data-parallel over the batch.

Full inputs [32, 8732, ...] come in; each core processes 4 batch rows.
Per core the 4x8732 anchors are laid out flat across 128 partitions as
4 row-blocks of 32 partitions x 273 anchor-"groups" (32*273 = 8736, the
last 4 slots of each row-block's final partition are padding).

Device computes, per core:
  - sumexp / lse per anchor (ACT exp + DVE reduce over the 81 classes)
  - pos mask / pos count (from actual_bbox_deltas)
  - Huber localization sum over positives
  - S1 = sum(actual_labels * pred_labels)   (per-chunk accumulators)
  - S2 = sum_pos lse, S3 = sum_pos pred0, S4 = sum_all pred0
  - hard-negative top-k sum via an 11-step vectorized binary search on the
    threshold t_r per batch row (k_r = 3*pos_count_r), using
    sum_topk = sum(v*[v>t]) + t*(k - count(v>t))
Host combines the 8 cores' scalar partials and performs the final division.

One-hot-label identity used:  conf = lse - dot(labels, pred);  for negative
anchors dot = pred[:, 0], so
  sum_pos conf = S2 - (S1 - (S4 - S3)).

DMA: pred_labels + bbox stream on the Activation HWDGE queue, actual_labels
on the Sync HWDGE queue, so the two big streams issue descriptors in
parallel. The mining binary search runs entirely on ACT+PE so the DVE
stream (sumexp reduces + label dots) is never blocked by it.
"""

import numpy as np

import concourse.bass as bass
import concourse.bacc as bacc
import concourse.tile as tile
import concourse.mybir as mybir
from concourse.bass_utils import run_bass_kernel_spmd

F32 = mybir.dt.float32
AX = mybir.AxisListType
OP = mybir.AluOpType
AF = mybir.ActivationFunctionType

B, P, C = 32, 8732, 81
NCORES = 8
BL = B // NCORES            # batch rows per core = 4
PPR = 32                    # partitions per row-block
G = 273                     # anchor groups per partition (32*273 = 8736)
NV = P - (PPR - 1) * G      # valid groups on last partition of a block = 269
NEG_BIG = -1.0e30
NITER = 11                  # binary-search iterations (range [0, 32))
T0 = 16.0
NF = 32                     # output partial columns

CHP = 91                    # pred DMA chunk (3 chunks)
NCHP = G // CHP
CHL = 39                    # labels DMA chunk (7 chunks) = compute sub-chunk
NCHL = G // CHL

# column map of the [1, NF] per-core output
COL_LOC, COL_S2, COL_S3, COL_S4, COL_POS, COL_NEG = 0, 1, 2, 3, 4, 5
COL_S1 = 8                  # .. COL_S1 + NCHL - 1


def _dma_grid(nc, eng, dst, dram, inner, g0, g1, padz):
    """Fill dst[0:128, 0:(g1-g0), :inner] from dram [BL, P, inner] where
    partition 32*r+q holds groups [q*G + g0, q*G + g1) of row r. Handles
    the ragged tail (groups >= NV invalid on the last partition of each
    block) with separate DMAs + a pad fill."""
    gv = min(g1, NV)        # groups valid on every partition
    # main body: all 128 partitions, groups [g0, gv)
    src = bass.AP(dram, g0 * inner,
                  [[P * inner, BL], [G * inner, PPR], [inner, gv - g0],
                   [1, inner]])
    eng.dma_start(dst[:, 0:gv - g0, :], src)
    if g1 > NV:
        # groups [NV, g1) valid only on partitions q < 31 of each block
        for r in range(BL):
            p0 = r * PPR
            src = bass.AP(dram, r * P * inner + NV * inner,
                          [[G * inner, PPR - 1], [inner, g1 - NV], [1, inner]])
            eng.dma_start(dst[p0:p0 + PPR - 1, NV - g0:g1 - g0, :], src)
            eng.dma_start(dst[p0 + PPR - 1:p0 + PPR, NV - g0:g1 - g0, :],
                          padz[:, 0:g1 - NV, 0:inner])


def build():
    nc = bacc.Bacc("TRN2", target_bir_lowering=False, debug=False)

    d_ab = nc.dram_tensor("actual_bbox_deltas", [BL, P, 4], F32, kind="ExternalInput")
    d_al = nc.dram_tensor("actual_labels", [BL, P, C], F32, kind="ExternalInput")
    d_pb = nc.dram_tensor("pred_bbox_deltas", [BL, P, 4], F32, kind="ExternalInput")
    d_pl = nc.dram_tensor("pred_labels", [BL, P, C], F32, kind="ExternalInput")
    d_out = nc.dram_tensor("out", [1, NF], F32, kind="ExternalOutput")

    with tile.TileContext(nc) as tc:
        with (
            tc.tile_pool(name="const", bufs=1) as constp,
            tc.tile_pool(name="resident", bufs=1) as resp,
            tc.tile_pool(name="bbox", bufs=1) as bbp,
            tc.tile_pool(name="hub", bufs=1) as hubp,
            tc.tile_pool(name="expj", bufs=2) as expp,
            tc.tile_pool(name="lblchunk", bufs=2) as lblp,
            tc.tile_pool(name="small", bufs=2) as smallp,
            tc.tile_pool(name="mine", bufs=2) as minep,
            tc.tile_pool(name="psum", bufs=2, space="PSUM") as psump,
            tc.tile_pool(name="dram", bufs=1, space="DRAM") as dramp,
        ):
            # ---- pad-fill constant buffers (engine memsets cannot target
            # partition bases that aren't 32-aligned; pads are DMA-filled
            # from small internal-DRAM buffers instead) ----
            stg = constp.tile([128, 4, C], F32, tag="stg")
            nc.gpsimd.memset(stg[:, :, :], 0.0)
            stageneg = constp.tile([128, 4], F32, tag="stageneg")
            nc.gpsimd.memset(stageneg[:, :], NEG_BIG)
            padz = dramp.tile([1, 4, C], F32, tag="padz")
            nc.gpsimd.dma_start(padz[:, :, :], stg[0:1, :, :])
            padneg = dramp.tile([1, 4], F32, tag="padneg")
            nc.gpsimd.dma_start(padneg[:, :], stageneg[0:1, :])

            # ---- constants ----
            blockones = constp.tile([128, 128], F32)
            nc.gpsimd.memset(blockones[:, :], 0.0)
            for r in range(BL):
                nc.gpsimd.memset(
                    blockones[r * PPR:(r + 1) * PPR, r * PPR:(r + 1) * PPR], 1.0)
            onescol = constp.tile([128, 1], F32)
            nc.gpsimd.memset(onescol[:, :], 1.0)
            fpart = constp.tile([128, NF], F32)
            nc.gpsimd.memset(fpart[:, :], 0.0)
            negone = constp.tile([128, 1], F32)
            nc.gpsimd.memset(negone[:, :], -1.0)

            # ---- bbox + pred DMAs (Activation HWDGE queue) ----
            abt = bbp.tile([128, G, 4], F32, tag="abt")
            pbt = bbp.tile([128, G, 4], F32, tag="pbt")
            _dma_grid(nc, nc.gpsimd, abt, d_ab, 4, 0, G, padz)
            _dma_grid(nc, nc.gpsimd, pbt, d_pb, 4, 0, G, padz)

            # pred first on the Sync HWDGE queue; the labels DMAs are
            # enqueued behind it (FIFO) so pred streams at full rate and
            # the mining chain can start as early as possible.
            pred = resp.tile([128, G, C], F32, tag="pred")
            for k in range(NCHP):
                _dma_grid(nc, nc.sync, pred[:, k * CHP:(k + 1) * CHP, :],
                          d_pl, C, k * CHP, (k + 1) * CHP, padz)

            # ---- bbox compute ----
            absmax = bbp.tile([128, G], F32, tag="absmax")
            nc.vector.tensor_reduce(absmax[:, :], abt[:, :, :], AX.X, OP.max,
                                    apply_absolute_value=True)
            posmask = bbp.tile([128, G], F32, tag="posmask")
            nc.vector.tensor_scalar(posmask[:, :], absmax[:, :], 0.0, None, OP.is_gt)

            pospart = bbp.tile([128, 1], F32, tag="pospart")
            nc.vector.tensor_reduce(pospart[:, :], posmask[:, :], AX.X, OP.add)
            nc.vector.tensor_copy(fpart[:, COL_POS:COL_POS + 1], pospart[:, :])
            pos_rep = psump.tile([128, 1], F32, tag="posrep")
            nc.tensor.matmul(pos_rep[:, :], blockones[:, :], pospart[:, :])
            # kcol = 3*pos ; sign-count threshold negk2 = n_tot - 6*pos
            kcol = bbp.tile([128, 1], F32, tag="kcol")
            nc.vector.tensor_scalar(kcol[:, :], pos_rep[:, :], 3.0, None, OP.mult)
            negk2 = bbp.tile([128, 1], F32, tag="negk2")
            nc.vector.tensor_scalar(negk2[:, :], pos_rep[:, :], -6.0,
                                    float(PPR * G), OP.mult, OP.add)

            # Huber loc loss (3 scratch tiles, in-place chains)
            dt_ = hubp.tile([128, G, 4], F32, tag="hd")
            nc.vector.tensor_sub(dt_[:, :, :], pbt[:, :, :], abt[:, :, :])
            nc.scalar.activation(dt_[:, :, :], dt_[:, :, :], AF.Abs)  # a = |d|
            mt = hubp.tile([128, G, 4], F32, tag="hm")
            nc.vector.tensor_single_scalar(mt[:, :, :], dt_[:, :, :], 1.0, OP.min)
            st = hubp.tile([128, G, 4], F32, tag="hs")
            nc.scalar.activation(st[:, :, :], mt[:, :, :], AF.Square,
                                 scale=float(np.sqrt(0.5)))       # 0.5*m^2
            nc.scalar.activation(dt_[:, :, :], dt_[:, :, :], AF.Relu,
                                 bias=negone[:, :])               # relu(a-1)
            nc.vector.tensor_add(st[:, :, :], st[:, :, :], dt_[:, :, :])
            hpb = hubp.tile([128, G], F32, tag="hpb")
            nc.vector.tensor_reduce(hpb[:, :], st[:, :, :], AX.X, OP.add)
            hjunk = hubp.tile([128, G], F32, tag="hjunk")
            nc.vector.scalar_tensor_tensor(
                hjunk[:, :], hpb[:, :], 0.25, posmask[:, :], OP.mult, OP.mult,
                accum_out=fpart[:, COL_LOC:COL_LOC + 1])

            # ---- exp + per-anchor sumexp over pred (sub-chunks of CHL) ----
            sumexp = resp.tile([128, G], F32, tag="sumexp")
            for k in range(NCHL):
                sl = pred[:, k * CHL:(k + 1) * CHL, :]
                ex = expp.tile([128, CHL, C], F32, tag="exp")
                nc.scalar.activation(ex[:, :, :], sl, AF.Exp)
                nc.vector.tensor_reduce(sumexp[:, k * CHL:(k + 1) * CHL],
                                        ex[:, :, :], AX.X, OP.add)

            lse = resp.tile([128, G], F32, tag="lse")
            nc.scalar.activation(lse[:, :], sumexp[:, :], AF.Ln)
            pred0 = pred[:, :, 0]
            nconf = resp.tile([128, G], F32, tag="nconf")
            nc.vector.tensor_sub(nconf[:, :], lse[:, :], pred0)
            masked = resp.tile([128, G], F32, tag="masked")
            nc.vector.scalar_tensor_tensor(
                masked[:, :], posmask[:, :], NEG_BIG, nconf[:, :], OP.mult, OP.add)
            for r in range(BL):
                nc.gpsimd.dma_start(
                    masked[r * PPR + PPR - 1:r * PPR + PPR, NV:G], padneg[:, :])

            # S2, S3, S4
            j2 = smallp.tile([128, G], F32, tag="sjunk")
            nc.vector.scalar_tensor_tensor(
                j2[:, :], posmask[:, :], 0.0, lse[:, :], OP.bypass, OP.mult,
                accum_out=fpart[:, COL_S2:COL_S2 + 1])
            j3 = smallp.tile([128, G], F32, tag="sjunk")
            nc.vector.scalar_tensor_tensor(
                j3[:, :], posmask[:, :], 0.0, pred0, OP.bypass, OP.mult,
                accum_out=fpart[:, COL_S3:COL_S3 + 1])
            nc.vector.tensor_reduce(fpart[:, COL_S4:COL_S4 + 1], pred0, AX.X, OP.add)

            # ---- hard-negative mining: binary search on t per row (ACT+PE
            # only, so the DVE stream of label dots is never blocked) ----
            negt = minep.tile([128, 1], F32, tag="negt")
            nc.gpsimd.memset(negt[:, :], -T0)
            for i in range(NITER):
                cjunk = minep.tile([128, G], F32, tag="cjunk")
                cnt = minep.tile([128, 1], F32, tag="cnt")
                # sum(sign(masked - t)) = cnt_gt - cnt_le   (per partition)
                nc.scalar.activation(cjunk[:, :], masked[:, :], AF.Sign,
                                     bias=negt[:, :], accum_out=cnt[:, :])
                srep = psump.tile([128, 1], F32, tag="srep")
                nc.tensor.matmul(srep[:, :], blockones[:, :], cnt[:, :])
                # s = sign(sum_rep - (2k - n)) : +1 -> count>k -> t too low
                sdir = minep.tile([128, 1], F32, tag="sdir")
                nc.scalar.activation(sdir[:, :], srep[:, :], AF.Sign,
                                     bias=negk2[:, :])
                delta = T0 / (2 ** (i + 1))
                negt2 = minep.tile([128, 1], F32, tag="negt")
                nc.scalar.activation(negt2[:, :], sdir[:, :], AF.Identity,
                                     bias=negt[:, :], scale=-delta)
                negt = negt2

            # ---- actual_labels stream (Sync HWDGE queue) + dot products ----
            for k in range(NCHL):
                lbl = lblp.tile([128, CHL, C], F32, tag="lbl")
                _dma_grid(nc, nc.sync, lbl, d_al, C, k * CHL, (k + 1) * CHL, padz)
                dj = lblp.tile([128, CHL, C], F32, tag="dotjunk")
                nc.vector.scalar_tensor_tensor(
                    dj[:, :, :], lbl[:, :, :], 0.0,
                    pred[:, k * CHL:(k + 1) * CHL, :], OP.bypass, OP.mult,
                    accum_out=fpart[:, COL_S1 + k:COL_S1 + k + 1])

            # final mining pass (emitted after the label dots so these
            # mining-gated ops sit at the tail of the in-order DVE stream)
            tcol = minep.tile([128, 1], F32, tag="tcol")
            nc.vector.tensor_scalar(tcol[:, :], negt[:, :], -1.0, None, OP.mult)
            fjunk = minep.tile([128, G], F32, tag="fjunk")
            cntf = minep.tile([128, 1], F32, tag="cntf")
            nc.vector.tensor_scalar(fjunk[:, :], masked[:, :], tcol[:, :], 0.0,
                                    OP.is_gt, OP.add, accum_out=cntf[:, :])
            fjunk2 = minep.tile([128, G], F32, tag="fjunk")
            negsump = minep.tile([128, 1], F32, tag="negsump")
            nc.vector.scalar_tensor_tensor(
                fjunk2[:, :], masked[:, :], tcol[:, :], masked[:, :],
                OP.is_gt, OP.mult, accum_out=negsump[:, :])
            # contrib = negsum - t*cntf + t*kcol/PPR
            c1 = minep.tile([128, 1], F32, tag="c1")
            nc.vector.tensor_mul(c1[:, :], tcol[:, :], cntf[:, :])
            d1 = minep.tile([128, 1], F32, tag="d1")
            nc.vector.scalar_tensor_tensor(
                d1[:, :], kcol[:, :], 1.0 / PPR, tcol[:, :], OP.mult, OP.mult)
            e1 = minep.tile([128, 1], F32, tag="e1")
            nc.vector.tensor_sub(e1[:, :], negsump[:, :], c1[:, :])
            nc.vector.tensor_add(fpart[:, COL_NEG:COL_NEG + 1], e1[:, :], d1[:, :])

            # ---- final cross-partition reduce and output ----
            opsum = psump.tile([1, NF], F32, tag="opsum")
            nc.tensor.matmul(opsum[:, :], onescol[:, :], fpart[:, :])
            osb = constp.tile([1, NF], F32)
            nc.vector.tensor_copy(osb[:, :], opsum[:, :])
            nc.sync.dma_start(d_out[:, :], osb[:, :])

    nc.compile()
    return nc


_nc = None


def kernel(actual_bbox_deltas, actual_labels, pred_bbox_deltas, pred_labels):
    global _nc
    if _nc is None:
        _nc = build()

    in_maps = []
    for core in range(NCORES):
        r0 = core * BL
        in_maps.append({
            "actual_bbox_deltas": np.ascontiguousarray(
                actual_bbox_deltas[r0:r0 + BL], np.float32),
            "actual_labels": np.ascontiguousarray(
                actual_labels[r0:r0 + BL], np.float32),
            "pred_bbox_deltas": np.ascontiguousarray(
                pred_bbox_deltas[r0:r0 + BL], np.float32),
            "pred_labels": np.ascontiguousarray(
                pred_labels[r0:r0 + BL], np.float32),
        })

    res = run_bass_kernel_spmd(_nc, in_maps, core_ids=list(range(NCORES)))
    loc = conf = pos = 0.0
    for core in range(NCORES):
        o = res.results[core]["out"][0].astype(np.float64)
        s1 = o[COL_S1:COL_S1 + NCHL].sum()
        loc += o[COL_LOC]
        conf += o[COL_S2] - s1 + o[COL_S4] - o[COL_S3] + o[COL_NEG]
        pos += o[COL_POS]
    if pos == 0:
        return (np.float32(0.0), np.float32(0.0))
    return (np.float32(loc / pos), np.float32(conf / pos))


# revision 19
# speedup vs baseline: 2.1061x; 1.7392x over previous
"""SSD-style multibox loss (Huber loc + softmax conf with hard-negative
mining) on 8 Trainium2 NeuronCores, pure data-parallel over the batch.

Full inputs [32, 8732, ...] come in; each core processes 4 batch rows.
Per core the 4x8732 anchors are laid out flat across 128 partitions as
4 row-blocks of 32 partitions x 273 anchor-"groups" (32*273 = 8736, the
last 4 slots of each row-block's final partition are padding).

Device computes, per core:
  - sumexp / lse per anchor (ACT exp + DVE reduce over the 81 classes)
  - pos mask / pos count (from actual_bbox_deltas)
  - Huber localization sum over positives
  - S1 = sum(actual_labels * pred_labels)   (per-chunk accumulators)
  - S2 = sum_pos lse, S3 = sum_pos pred0, S4 = sum_all pred0
  - hard-negative top-k sum via an 11-step vectorized binary search on the
    threshold t_r per batch row (k_r = 3*pos_count_r), using
    sum_topk = sum(v*[v>t]) + t*(k - count(v>t))
Host combines the 8 cores' scalar partials and performs the final division.

One-hot-label identity used:  conf = lse - dot(labels, pred);  for negative
anchors dot = pred[:, 0], so
  sum_pos conf = S2 - (S1 - (S4 - S3)).

DMA: pred_labels + bbox stream on the Activation HWDGE queue, actual_labels
on the Sync HWDGE queue, so the two big streams issue descriptors in
parallel. The mining binary search runs entirely on ACT+PE so the DVE
stream (sumexp reduces + label dots) is never blocked by it.
"""

import numpy as np

import concourse.bass as bass
import concourse.bacc as bacc
import concourse.tile as tile
import concourse.mybir as mybir
from concourse.bass_utils import run_bass_kernel_spmd

F32 = mybir.dt.float32
AX = mybir.AxisListType
OP = mybir.AluOpType
AF = mybir.ActivationFunctionType

B, P, C = 32, 8732, 81
NCORES = 8
BL = B // NCORES            # batch rows per core = 4
PPR = 32                    # partitions per row-block
G = 273                     # anchor groups per partition (32*273 = 8736)
NV = P - (PPR - 1) * G      # valid groups on last partition of a block = 269
NEG_BIG = -1.0e30
NITER = 11                  # binary-search iterations (range [0, 32))
T0 = 16.0
NF = 32                     # output partial columns

CHP = 91                    # pred DMA chunk (3 chunks)
NCHP = G // CHP
CHL = 39                    # labels DMA chunk (7 chunks) = compute sub-chunk
NCHL = G // CHL

# column map of the [1, NF] per-core output
COL_LOC, COL_S2, COL_S3, COL_S4, COL_POS, COL_NEG = 0, 1, 2, 3, 4, 5
COL_S1 = 8                  # .. COL_S1 + NCHL - 1


def _dma_grid(nc, eng, dst, dram, inner, g0, g1, padz):
    """Fill dst[0:128, 0:(g1-g0), :inner] from dram [BL, P, inner] where
    partition 32*r+q holds groups [q*G + g0, q*G + g1) of row r. One 3D
    DMA per row-block (merged 4D patterns fragment descriptors and issue
    ~6x slower). Handles the ragged tail (groups >= NV invalid on the
    last partition of each block) with separate DMAs + a pad fill."""
    gv = min(g1, NV)        # groups valid on every partition
    for r in range(BL):
        p0 = r * PPR
        src = bass.AP(dram, r * P * inner + g0 * inner,
                      [[G * inner, PPR], [inner, gv - g0], [1, inner]])
        eng.dma_start(dst[p0:p0 + PPR, 0:gv - g0, :], src)
    if g1 > NV:
        # groups [NV, g1) valid only on partitions q < 31 of each block
        for r in range(BL):
            p0 = r * PPR
            src = bass.AP(dram, r * P * inner + NV * inner,
                          [[G * inner, PPR - 1], [inner, g1 - NV], [1, inner]])
            eng.dma_start(dst[p0:p0 + PPR - 1, NV - g0:g1 - g0, :], src)
            eng.dma_start(dst[p0 + PPR - 1:p0 + PPR, NV - g0:g1 - g0, :],
                          padz[:, 0:g1 - NV, 0:inner])


def build():
    nc = bacc.Bacc("TRN2", target_bir_lowering=False, debug=False)

    d_ab = nc.dram_tensor("actual_bbox_deltas", [BL, P, 4], F32, kind="ExternalInput")
    d_al = nc.dram_tensor("actual_labels", [BL, P, C], F32, kind="ExternalInput")
    d_pb = nc.dram_tensor("pred_bbox_deltas", [BL, P, 4], F32, kind="ExternalInput")
    d_pl = nc.dram_tensor("pred_labels", [BL, P, C], F32, kind="ExternalInput")
    d_out = nc.dram_tensor("out", [1, NF], F32, kind="ExternalOutput")

    with tile.TileContext(nc) as tc:
        with (
            tc.tile_pool(name="const", bufs=1) as constp,
            tc.tile_pool(name="resident", bufs=1) as resp,
            tc.tile_pool(name="bbox", bufs=1) as bbp,
            tc.tile_pool(name="hub", bufs=1) as hubp,
            tc.tile_pool(name="expj", bufs=2) as expp,
            tc.tile_pool(name="lblchunk", bufs=2) as lblp,
            tc.tile_pool(name="small", bufs=2) as smallp,
            tc.tile_pool(name="mine", bufs=2) as minep,
            tc.tile_pool(name="psum", bufs=2, space="PSUM") as psump,
            tc.tile_pool(name="dram", bufs=1, space="DRAM") as dramp,
        ):
            # ---- pad-fill constant buffers (engine memsets cannot target
            # partition bases that aren't 32-aligned; pads are DMA-filled
            # from small internal-DRAM buffers instead) ----
            stg = constp.tile([128, 4, C], F32, tag="stg")
            nc.gpsimd.memset(stg[:, :, :], 0.0)
            stageneg = constp.tile([128, 4], F32, tag="stageneg")
            nc.gpsimd.memset(stageneg[:, :], NEG_BIG)
            padz = dramp.tile([1, 4, C], F32, tag="padz")
            nc.gpsimd.dma_start(padz[:, :, :], stg[0:1, :, :])
            padneg = dramp.tile([1, 4], F32, tag="padneg")
            nc.gpsimd.dma_start(padneg[:, :], stageneg[0:1, :])

            # ---- constants ----
            blockones = constp.tile([128, 128], F32)
            nc.gpsimd.memset(blockones[:, :], 0.0)
            for r in range(BL):
                nc.gpsimd.memset(
                    blockones[r * PPR:(r + 1) * PPR, r * PPR:(r + 1) * PPR], 1.0)
            onescol = constp.tile([128, 1], F32)
            nc.gpsimd.memset(onescol[:, :], 1.0)
            fpart = constp.tile([128, NF], F32)
            nc.gpsimd.memset(fpart[:, :], 0.0)
            negone = constp.tile([128, 1], F32)
            nc.gpsimd.memset(negone[:, :], -1.0)

            # ---- bbox + pred DMAs (Activation HWDGE queue) ----
            abt = bbp.tile([128, G, 4], F32, tag="abt")
            pbt = bbp.tile([128, G, 4], F32, tag="pbt")
            _dma_grid(nc, nc.gpsimd, abt, d_ab, 4, 0, G, padz)
            _dma_grid(nc, nc.gpsimd, pbt, d_pb, 4, 0, G, padz)

            # pred first on the Sync HWDGE queue; the labels DMAs are
            # enqueued behind it (FIFO) so pred streams at full rate and
            # the mining chain can start as early as possible.
            pred = resp.tile([128, G, C], F32, tag="pred")
            for k in range(NCHP):
                _dma_grid(nc, nc.sync, pred[:, k * CHP:(k + 1) * CHP, :],
                          d_pl, C, k * CHP, (k + 1) * CHP, padz)

            # ---- bbox compute ----
            absmax = bbp.tile([128, G], F32, tag="absmax")
            nc.vector.tensor_reduce(absmax[:, :], abt[:, :, :], AX.X, OP.max,
                                    apply_absolute_value=True)
            posmask = bbp.tile([128, G], F32, tag="posmask")
            nc.vector.tensor_scalar(posmask[:, :], absmax[:, :], 0.0, None, OP.is_gt)

            pospart = bbp.tile([128, 1], F32, tag="pospart")
            nc.vector.tensor_reduce(pospart[:, :], posmask[:, :], AX.X, OP.add)
            nc.vector.tensor_copy(fpart[:, COL_POS:COL_POS + 1], pospart[:, :])
            pos_rep = psump.tile([128, 1], F32, tag="posrep")
            nc.tensor.matmul(pos_rep[:, :], blockones[:, :], pospart[:, :])
            # kcol = 3*pos ; sign-count threshold negk2 = n_tot - 6*pos
            kcol = bbp.tile([128, 1], F32, tag="kcol")
            nc.vector.tensor_scalar(kcol[:, :], pos_rep[:, :], 3.0, None, OP.mult)
            negk2 = bbp.tile([128, 1], F32, tag="negk2")
            nc.vector.tensor_scalar(negk2[:, :], pos_rep[:, :], -6.0,
                                    float(PPR * G), OP.mult, OP.add)

            # Huber loc loss (3 scratch tiles, in-place chains)
            dt_ = hubp.tile([128, G, 4], F32, tag="hd")
            nc.vector.tensor_sub(dt_[:, :, :], pbt[:, :, :], abt[:, :, :])
            nc.scalar.activation(dt_[:, :, :], dt_[:, :, :], AF.Abs)  # a = |d|
            mt = hubp.tile([128, G, 4], F32, tag="hm")
            nc.vector.tensor_single_scalar(mt[:, :, :], dt_[:, :, :], 1.0, OP.min)
            st = hubp.tile([128, G, 4], F32, tag="hs")
            nc.scalar.activation(st[:, :, :], mt[:, :, :], AF.Square,
                                 scale=float(np.sqrt(0.5)))       # 0.5*m^2
            nc.scalar.activation(dt_[:, :, :], dt_[:, :, :], AF.Relu,
                                 bias=negone[:, :])               # relu(a-1)
            nc.vector.tensor_add(st[:, :, :], st[:, :, :], dt_[:, :, :])
            hpb = hubp.tile([128, G], F32, tag="hpb")
            nc.vector.tensor_reduce(hpb[:, :], st[:, :, :], AX.X, OP.add)
            hjunk = hubp.tile([128, G], F32, tag="hjunk")
            nc.vector.scalar_tensor_tensor(
                hjunk[:, :], hpb[:, :], 0.25, posmask[:, :], OP.mult, OP.mult,
                accum_out=fpart[:, COL_LOC:COL_LOC + 1])

            # ---- exp + per-anchor sumexp over pred (sub-chunks of CHL) ----
            sumexp = resp.tile([128, G], F32, tag="sumexp")
            for k in range(NCHL):
                sl = pred[:, k * CHL:(k + 1) * CHL, :]
                ex = expp.tile([128, CHL, C], F32, tag="exp")
                nc.scalar.activation(ex[:, :, :], sl, AF.Exp)
                nc.vector.tensor_reduce(sumexp[:, k * CHL:(k + 1) * CHL],
                                        ex[:, :, :], AX.X, OP.add)

            lse = resp.tile([128, G], F32, tag="lse")
            nc.scalar.activation(lse[:, :], sumexp[:, :], AF.Ln)
            pred0 = pred[:, :, 0]
            nconf = resp.tile([128, G], F32, tag="nconf")
            nc.vector.tensor_sub(nconf[:, :], lse[:, :], pred0)
            masked = resp.tile([128, G], F32, tag="masked")
            nc.vector.scalar_tensor_tensor(
                masked[:, :], posmask[:, :], NEG_BIG, nconf[:, :], OP.mult, OP.add)
            for r in range(BL):
                nc.gpsimd.dma_start(
                    masked[r * PPR + PPR - 1:r * PPR + PPR, NV:G], padneg[:, :])

            # S2, S3, S4
            j2 = smallp.tile([128, G], F32, tag="sjunk")
            nc.vector.scalar_tensor_tensor(
                j2[:, :], posmask[:, :], 0.0, lse[:, :], OP.bypass, OP.mult,
                accum_out=fpart[:, COL_S2:COL_S2 + 1])
            j3 = smallp.tile([128, G], F32, tag="sjunk")
            nc.vector.scalar_tensor_tensor(
                j3[:, :], posmask[:, :], 0.0, pred0, OP.bypass, OP.mult,
                accum_out=fpart[:, COL_S3:COL_S3 + 1])
            nc.vector.tensor_reduce(fpart[:, COL_S4:COL_S4 + 1], pred0, AX.X, OP.add)

            # ---- hard-negative mining: binary search on t per row (ACT+PE
            # only, so the DVE stream of label dots is never blocked) ----
            negt = minep.tile([128, 1], F32, tag="negt")
            nc.gpsimd.memset(negt[:, :], -T0)
            for i in range(NITER):
                cjunk = minep.tile([128, G], F32, tag="cjunk")
                cnt = minep.tile([128, 1], F32, tag="cnt")
                # sum(sign(masked - t)) = cnt_gt - cnt_le   (per partition)
                nc.scalar.activation(cjunk[:, :], masked[:, :], AF.Sign,
                                     bias=negt[:, :], accum_out=cnt[:, :])
                srep = psump.tile([128, 1], F32, tag="srep")
                nc.tensor.matmul(srep[:, :], blockones[:, :], cnt[:, :])
                # s = sign(sum_rep - (2k - n)) : +1 -> count>k -> t too low
                sdir = minep.tile([128, 1], F32, tag="sdir")
                nc.scalar.activation(sdir[:, :], srep[:, :], AF.Sign,
                                     bias=negk2[:, :])
                delta = T0 / (2 ** (i + 1))
                negt2 = minep.tile([128, 1], F32, tag="negt")
                nc.scalar.activation(negt2[:, :], sdir[:, :], AF.Identity,
                                     bias=negt[:, :], scale=-delta)
                negt = negt2

            # ---- actual_labels stream (Sync HWDGE queue) + dot products ----
            for k in range(NCHL):
                lbl = lblp.tile([128, CHL, C], F32, tag="lbl")
                _dma_grid(nc, nc.sync, lbl, d_al, C, k * CHL, (k + 1) * CHL, padz)
                dj = lblp.tile([128, CHL, C], F32, tag="dotjunk")
                nc.vector.scalar_tensor_tensor(
                    dj[:, :, :], lbl[:, :, :], 0.0,
                    pred[:, k * CHL:(k + 1) * CHL, :], OP.bypass, OP.mult,
                    accum_out=fpart[:, COL_S1 + k:COL_S1 + k + 1])

            # final mining pass (emitted after the label dots so these
            # mining-gated ops sit at the tail of the in-order DVE stream)
            tcol = minep.tile([128, 1], F32, tag="tcol")
            nc.vector.tensor_scalar(tcol[:, :], negt[:, :], -1.0, None, OP.mult)
            fjunk = minep.tile([128, G], F32, tag="fjunk")
            cntf = minep.tile([128, 1], F32, tag="cntf")
            nc.vector.tensor_scalar(fjunk[:, :], masked[:, :], tcol[:, :], 0.0,
                                    OP.is_gt, OP.add, accum_out=cntf[:, :])
            fjunk2 = minep.tile([128, G], F32, tag="fjunk")
            negsump = minep.tile([128, 1], F32, tag="negsump")
            nc.vector.scalar_tensor_tensor(
                fjunk2[:, :], masked[:, :], tcol[:, :], masked[:, :],
                OP.is_gt, OP.mult, accum_out=negsump[:, :])
            # contrib = negsum - t*cntf + t*kcol/PPR
            c1 = minep.tile([128, 1], F32, tag="c1")
            nc.vector.tensor_mul(c1[:, :], tcol[:, :], cntf[:, :])
            d1 = minep.tile([128, 1], F32, tag="d1")
            nc.vector.scalar_tensor_tensor(
                d1[:, :], kcol[:, :], 1.0 / PPR, tcol[:, :], OP.mult, OP.mult)
            e1 = minep.tile([128, 1], F32, tag="e1")
            nc.vector.tensor_sub(e1[:, :], negsump[:, :], c1[:, :])
            nc.vector.tensor_add(fpart[:, COL_NEG:COL_NEG + 1], e1[:, :], d1[:, :])

            # ---- final cross-partition reduce and output ----
            opsum = psump.tile([1, NF], F32, tag="opsum")
            nc.tensor.matmul(opsum[:, :], onescol[:, :], fpart[:, :])
            osb = constp.tile([1, NF], F32)
            nc.vector.tensor_copy(osb[:, :], opsum[:, :])
            nc.sync.dma_start(d_out[:, :], osb[:, :])

    nc.compile()
    return nc


_nc = None


def kernel(actual_bbox_deltas, actual_labels, pred_bbox_deltas, pred_labels):
    global _nc
    if _nc is None:
        _nc = build()

    in_maps = []
    for core in range(NCORES):
        r0 = core * BL
        in_maps.append({
            "actual_bbox_deltas": np.ascontiguousarray(
                actual_bbox_deltas[r0:r0 + BL], np.float32),
            "actual_labels": np.ascontiguousarray(
                actual_labels[r0:r0 + BL], np.float32),
            "pred_bbox_deltas": np.ascontiguousarray(
                pred_bbox_deltas[r0:r0 + BL], np.float32),
            "pred_labels": np.ascontiguousarray(
                pred_labels[r0:r0 + BL], np.float32),
        })

    res = run_bass_kernel_spmd(_nc, in_maps, core_ids=list(range(NCORES)))
    loc = conf = pos = 0.0
    for core in range(NCORES):
        o = res.results[core]["out"][0].astype(np.float64)
        s1 = o[COL_S1:COL_S1 + NCHL].sum()
        loc += o[COL_LOC]
        conf += o[COL_S2] - s1 + o[COL_S4] - o[COL_S3] + o[COL_NEG]
        pos += o[COL_POS]
    if pos == 0:
        return (np.float32(0.0), np.float32(0.0))
    return (np.float32(loc / pos), np.float32(conf / pos))


# revision 20
# speedup vs baseline: 2.3120x; 1.0978x over previous
"""SSD-style multibox loss (Huber loc + softmax conf with hard-negative
mining) on 8 Trainium2 NeuronCores, pure data-parallel over the batch.

Full inputs [32, 8732, ...] come in; each core processes 4 batch rows.
Per core the 4x8732 anchors are laid out flat across 128 partitions as
4 row-blocks of 32 partitions x 273 anchor-"groups" (32*273 = 8736, the
last 4 slots of each row-block's final partition are padding).

Device computes, per core:
  - sumexp / lse per anchor (ACT exp + DVE reduce over the 81 classes)
  - pos mask / pos count (from actual_bbox_deltas)
  - Huber localization sum over positives
  - S1 = sum(actual_labels * pred_labels)   (per-chunk accumulators)
  - S2 = sum_pos lse, S3 = sum_pos pred0, S4 = sum_all pred0
  - hard-negative top-k sum via an 11-step vectorized binary search on the
    threshold t_r per batch row (k_r = 3*pos_count_r), using
    sum_topk = sum(v*[v>t]) + t*(k - count(v>t))
Host combines the 8 cores' scalar partials and performs the final division.

One-hot-label identity used:  conf = lse - dot(labels, pred);  for negative
anchors dot = pred[:, 0], so
  sum_pos conf = S2 - (S1 - (S4 - S3)).

DMA: pred_labels + bbox stream on the Activation HWDGE queue, actual_labels
on the Sync HWDGE queue, so the two big streams issue descriptors in
parallel. The mining binary search runs entirely on ACT+PE so the DVE
stream (sumexp reduces + label dots) is never blocked by it.
"""

import numpy as np

import concourse.bass as bass
import concourse.bacc as bacc
import concourse.tile as tile
import concourse.mybir as mybir
from concourse.bass_utils import run_bass_kernel_spmd

F32 = mybir.dt.float32
AX = mybir.AxisListType
OP = mybir.AluOpType
AF = mybir.ActivationFunctionType

B, P, C = 32, 8732, 81
NCORES = 8
BL = B // NCORES            # batch rows per core = 4
PPR = 32                    # partitions per row-block
G = 273                     # anchor groups per partition (32*273 = 8736)
NV = P - (PPR - 1) * G      # valid groups on last partition of a block = 269
NEG_BIG = -1.0e30
NITER = 11                  # binary-search iterations (range [0, 32))
T0 = 16.0
NF = 32                     # output partial columns

CHP = 91                    # pred DMA chunk (3 chunks)
NCHP = G // CHP
CHL = 39                    # labels DMA chunk (7 chunks) = compute sub-chunk
NCHL = G // CHL

# column map of the [1, NF] per-core output
COL_LOC, COL_S2, COL_S3, COL_S4, COL_POS, COL_NEG = 0, 1, 2, 3, 4, 5
COL_S1 = 8                  # .. COL_S1 + NCHL - 1


def _dma_grid(nc, eng, dst, dram, inner, g0, g1, padz):
    """Fill dst[0:128, 0:(g1-g0), :inner] from dram [BL, P, inner] where
    partition 32*r+q holds groups [q*G + g0, q*G + g1) of row r. One 3D
    DMA per row-block (merged 4D patterns fragment descriptors and issue
    ~6x slower). Handles the ragged tail (groups >= NV invalid on the
    last partition of each block) with separate DMAs + a pad fill."""
    gv = min(g1, NV)        # groups valid on every partition
    for r in range(BL):
        p0 = r * PPR
        src = bass.AP(dram, r * P * inner + g0 * inner,
                      [[G * inner, PPR], [inner, gv - g0], [1, inner]])
        eng.dma_start(dst[p0:p0 + PPR, 0:gv - g0, :], src)
    if g1 > NV:
        # groups [NV, g1) valid only on partitions q < 31 of each block
        for r in range(BL):
            p0 = r * PPR
            src = bass.AP(dram, r * P * inner + NV * inner,
                          [[G * inner, PPR - 1], [inner, g1 - NV], [1, inner]])
            eng.dma_start(dst[p0:p0 + PPR - 1, NV - g0:g1 - g0, :], src)
            eng.dma_start(dst[p0 + PPR - 1:p0 + PPR, NV - g0:g1 - g0, :],
                          padz[:, 0:g1 - NV, 0:inner])


def build():
    nc = bacc.Bacc("TRN2", target_bir_lowering=False, debug=False)

    d_ab = nc.dram_tensor("actual_bbox_deltas", [BL, P, 4], F32, kind="ExternalInput")
    d_al = nc.dram_tensor("actual_labels", [BL, P, C], F32, kind="ExternalInput")
    d_pb = nc.dram_tensor("pred_bbox_deltas", [BL, P, 4], F32, kind="ExternalInput")
    d_pl = nc.dram_tensor("pred_labels", [BL, P, C], F32, kind="ExternalInput")
    d_out = nc.dram_tensor("out", [1, NF], F32, kind="ExternalOutput")

    with tile.TileContext(nc) as tc:
        with (
            tc.tile_pool(name="const", bufs=1) as constp,
            tc.tile_pool(name="resident", bufs=1) as resp,
            tc.tile_pool(name="bbox", bufs=1) as bbp,
            tc.tile_pool(name="hub", bufs=1) as hubp,
            tc.tile_pool(name="expj", bufs=2) as expp,
            tc.tile_pool(name="lblchunk", bufs=2) as lblp,
            tc.tile_pool(name="small", bufs=2) as smallp,
            tc.tile_pool(name="mine", bufs=2) as minep,
            tc.tile_pool(name="psum", bufs=2, space="PSUM") as psump,
            tc.tile_pool(name="dram", bufs=1, space="DRAM") as dramp,
        ):
            # ---- pad-fill constant buffers (engine memsets cannot target
            # partition bases that aren't 32-aligned; pads are DMA-filled
            # from small internal-DRAM buffers instead) ----
            stg = constp.tile([128, 4, C], F32, tag="stg")
            nc.gpsimd.memset(stg[:, :, :], 0.0)
            stageneg = constp.tile([128, 4], F32, tag="stageneg")
            nc.gpsimd.memset(stageneg[:, :], NEG_BIG)
            padz = dramp.tile([1, 4, C], F32, tag="padz")
            nc.gpsimd.dma_start(padz[:, :, :], stg[0:1, :, :])
            padneg = dramp.tile([1, 4], F32, tag="padneg")
            nc.gpsimd.dma_start(padneg[:, :], stageneg[0:1, :])

            # ---- constants ----
            blockones = constp.tile([128, 128], F32)
            nc.gpsimd.memset(blockones[:, :], 0.0)
            for r in range(BL):
                nc.gpsimd.memset(
                    blockones[r * PPR:(r + 1) * PPR, r * PPR:(r + 1) * PPR], 1.0)
            onescol = constp.tile([128, 1], F32)
            nc.gpsimd.memset(onescol[:, :], 1.0)
            fpart = constp.tile([128, NF], F32)
            nc.gpsimd.memset(fpart[:, :], 0.0)
            negone = constp.tile([128, 1], F32)
            nc.gpsimd.memset(negone[:, :], -1.0)

            # ---- bbox + pred DMAs (Activation HWDGE queue) ----
            abt = bbp.tile([128, G, 4], F32, tag="abt")
            pbt = bbp.tile([128, G, 4], F32, tag="pbt")
            _dma_grid(nc, nc.gpsimd, abt, d_ab, 4, 0, G, padz)
            _dma_grid(nc, nc.gpsimd, pbt, d_pb, 4, 0, G, padz)

            # pred first on the Sync HWDGE queue; the labels DMAs are
            # enqueued behind it (FIFO) so pred streams at full rate and
            # the mining chain can start as early as possible.
            pred = resp.tile([128, G, C], F32, tag="pred")
            for k in range(NCHP):
                _dma_grid(nc, nc.gpsimd, pred[:, k * CHP:(k + 1) * CHP, :],
                          d_pl, C, k * CHP, (k + 1) * CHP, padz)

            # ---- bbox compute ----
            absmax = bbp.tile([128, G], F32, tag="absmax")
            nc.vector.tensor_reduce(absmax[:, :], abt[:, :, :], AX.X, OP.max,
                                    apply_absolute_value=True)
            posmask = bbp.tile([128, G], F32, tag="posmask")
            nc.vector.tensor_scalar(posmask[:, :], absmax[:, :], 0.0, None, OP.is_gt)

            pospart = bbp.tile([128, 1], F32, tag="pospart")
            nc.vector.tensor_reduce(pospart[:, :], posmask[:, :], AX.X, OP.add)
            nc.vector.tensor_copy(fpart[:, COL_POS:COL_POS + 1], pospart[:, :])
            pos_rep = psump.tile([128, 1], F32, tag="posrep")
            nc.tensor.matmul(pos_rep[:, :], blockones[:, :], pospart[:, :])
            # kcol = 3*pos ; sign-count threshold negk2 = n_tot - 6*pos
            kcol = bbp.tile([128, 1], F32, tag="kcol")
            nc.vector.tensor_scalar(kcol[:, :], pos_rep[:, :], 3.0, None, OP.mult)
            negk2 = bbp.tile([128, 1], F32, tag="negk2")
            nc.vector.tensor_scalar(negk2[:, :], pos_rep[:, :], -6.0,
                                    float(PPR * G), OP.mult, OP.add)

            # Huber loc loss (3 scratch tiles, in-place chains)
            dt_ = hubp.tile([128, G, 4], F32, tag="hd")
            nc.vector.tensor_sub(dt_[:, :, :], pbt[:, :, :], abt[:, :, :])
            nc.scalar.activation(dt_[:, :, :], dt_[:, :, :], AF.Abs)  # a = |d|
            mt = hubp.tile([128, G, 4], F32, tag="hm")
            nc.vector.tensor_single_scalar(mt[:, :, :], dt_[:, :, :], 1.0, OP.min)
            st = hubp.tile([128, G, 4], F32, tag="hs")
            nc.scalar.activation(st[:, :, :], mt[:, :, :], AF.Square,
                                 scale=float(np.sqrt(0.5)))       # 0.5*m^2
            nc.scalar.activation(dt_[:, :, :], dt_[:, :, :], AF.Relu,
                                 bias=negone[:, :])               # relu(a-1)
            nc.vector.tensor_add(st[:, :, :], st[:, :, :], dt_[:, :, :])
            hpb = hubp.tile([128, G], F32, tag="hpb")
            nc.vector.tensor_reduce(hpb[:, :], st[:, :, :], AX.X, OP.add)
            hjunk = hubp.tile([128, G], F32, tag="hjunk")
            nc.vector.scalar_tensor_tensor(
                hjunk[:, :], hpb[:, :], 0.25, posmask[:, :], OP.mult, OP.mult,
                accum_out=fpart[:, COL_LOC:COL_LOC + 1])

            # ---- exp + per-anchor sumexp over pred (sub-chunks of CHL) ----
            sumexp = resp.tile([128, G], F32, tag="sumexp")
            for k in range(NCHL):
                sl = pred[:, k * CHL:(k + 1) * CHL, :]
                ex = expp.tile([128, CHL, C], F32, tag="exp")
                nc.scalar.activation(ex[:, :, :], sl, AF.Exp)
                nc.vector.tensor_reduce(sumexp[:, k * CHL:(k + 1) * CHL],
                                        ex[:, :, :], AX.X, OP.add)

            lse = resp.tile([128, G], F32, tag="lse")
            nc.scalar.activation(lse[:, :], sumexp[:, :], AF.Ln)
            pred0 = pred[:, :, 0]
            nconf = resp.tile([128, G], F32, tag="nconf")
            nc.vector.tensor_sub(nconf[:, :], lse[:, :], pred0)
            masked = resp.tile([128, G], F32, tag="masked")
            nc.vector.scalar_tensor_tensor(
                masked[:, :], posmask[:, :], NEG_BIG, nconf[:, :], OP.mult, OP.add)
            for r in range(BL):
                nc.gpsimd.dma_start(
                    masked[r * PPR + PPR - 1:r * PPR + PPR, NV:G], padneg[:, :])

            # S2, S3, S4
            j2 = smallp.tile([128, G], F32, tag="sjunk")
            nc.vector.scalar_tensor_tensor(
                j2[:, :], posmask[:, :], 0.0, lse[:, :], OP.bypass, OP.mult,
                accum_out=fpart[:, COL_S2:COL_S2 + 1])
            j3 = smallp.tile([128, G], F32, tag="sjunk")
            nc.vector.scalar_tensor_tensor(
                j3[:, :], posmask[:, :], 0.0, pred0, OP.bypass, OP.mult,
                accum_out=fpart[:, COL_S3:COL_S3 + 1])
            nc.vector.tensor_reduce(fpart[:, COL_S4:COL_S4 + 1], pred0, AX.X, OP.add)

            # ---- hard-negative mining: binary search on t per row (ACT+PE
            # only, so the DVE stream of label dots is never blocked) ----
            negt = minep.tile([128, 1], F32, tag="negt")
            nc.gpsimd.memset(negt[:, :], -T0)
            for i in range(NITER):
                cjunk = minep.tile([128, G], F32, tag="cjunk")
                cnt = minep.tile([128, 1], F32, tag="cnt")
                # sum(sign(masked - t)) = cnt_gt - cnt_le   (per partition)
                nc.scalar.activation(cjunk[:, :], masked[:, :], AF.Sign,
                                     bias=negt[:, :], accum_out=cnt[:, :])
                srep = psump.tile([128, 1], F32, tag="srep")
                nc.tensor.matmul(srep[:, :], blockones[:, :], cnt[:, :])
                # s = sign(sum_rep - (2k - n)) : +1 -> count>k -> t too low
                sdir = minep.tile([128, 1], F32, tag="sdir")
                nc.scalar.activation(sdir[:, :], srep[:, :], AF.Sign,
                                     bias=negk2[:, :])
                delta = T0 / (2 ** (i + 1))
                negt2 = minep.tile([128, 1], F32, tag="negt")
                nc.scalar.activation(negt2[:, :], sdir[:, :], AF.Identity,
                                     bias=negt[:, :], scale=-delta)
                negt = negt2

            # ---- actual_labels stream (Sync HWDGE queue) + dot products ----
            for k in range(NCHL):
                lbl = lblp.tile([128, CHL, C], F32, tag="lbl")
                _dma_grid(nc, nc.sync, lbl, d_al, C, k * CHL, (k + 1) * CHL, padz)
                dj = lblp.tile([128, CHL, C], F32, tag="dotjunk")
                nc.vector.scalar_tensor_tensor(
                    dj[:, :, :], lbl[:, :, :], 0.0,
                    pred[:, k * CHL:(k + 1) * CHL, :], OP.bypass, OP.mult,
                    accum_out=fpart[:, COL_S1 + k:COL_S1 + k + 1])

            # final mining pass (emitted after the label dots so these
            # mining-gated ops sit at the tail of the in-order DVE stream)
            tcol = minep.tile([128, 1], F32, tag="tcol")
            nc.vector.tensor_scalar(tcol[:, :], negt[:, :], -1.0, None, OP.mult)
            fjunk = minep.tile([128, G], F32, tag="fjunk")
            cntf = minep.tile([128, 1], F32, tag="cntf")
            nc.vector.tensor_scalar(fjunk[:, :], masked[:, :], tcol[:, :], 0.0,
                                    OP.is_gt, OP.add, accum_out=cntf[:, :])
            fjunk2 = minep.tile([128, G], F32, tag="fjunk")
            negsump = minep.tile([128, 1], F32, tag="negsump")
            nc.vector.scalar_tensor_tensor(
                fjunk2[:, :], masked[:, :], tcol[:, :], masked[:, :],
                OP.is_gt, OP.mult, accum_out=negsump[:, :])
            # contrib = negsum - t*cntf + t*kcol/PPR
            c1 = minep.tile([128, 1], F32, tag="c1")
            nc.vector.tensor_mul(c1[:, :], tcol[:, :], cntf[:, :])
            d1 = minep.tile([128, 1], F32, tag="d1")
            nc.vector.scalar_tensor_tensor(
                d1[:, :], kcol[:, :], 1.0 / PPR, tcol[:, :], OP.mult, OP.mult)
            e1 = minep.tile([128, 1], F32, tag="e1")
            nc.vector.tensor_sub(e1[:, :], negsump[:, :], c1[:, :])
            nc.vector.tensor_add(fpart[:, COL_NEG:COL_NEG + 1], e1[:, :], d1[:, :])

            # ---- final cross-partition reduce and output ----
            opsum = psump.tile([1, NF], F32, tag="opsum")
            nc.tensor.matmul(opsum[:, :], onescol[:, :], fpart[:, :])
            osb = constp.tile([1, NF], F32)
            nc.vector.tensor_copy(osb[:, :], opsum[:, :])
            nc.sync.dma_start(d_out[:, :], osb[:, :])

    nc.compile()
    return nc


_nc = None


def kernel(actual_bbox_deltas, actual_labels, pred_bbox_deltas, pred_labels):
    global _nc
    if _nc is None:
        _nc = build()

    in_maps = []
    for core in range(NCORES):
        r0 = core * BL
        in_maps.append({
            "actual_bbox_deltas": np.ascontiguousarray(
                actual_bbox_deltas[r0:r0 + BL], np.float32),
            "actual_labels": np.ascontiguousarray(
                actual_labels[r0:r0 + BL], np.float32),
            "pred_bbox_deltas": np.ascontiguousarray(
                pred_bbox_deltas[r0:r0 + BL], np.float32),
            "pred_labels": np.ascontiguousarray(
                pred_labels[r0:r0 + BL], np.float32),
        })

    res = run_bass_kernel_spmd(_nc, in_maps, core_ids=list(range(NCORES)))
    loc = conf = pos = 0.0
    for core in range(NCORES):
        o = res.results[core]["out"][0].astype(np.float64)
        s1 = o[COL_S1:COL_S1 + NCHL].sum()
        loc += o[COL_LOC]
        conf += o[COL_S2] - s1 + o[COL_S4] - o[COL_S3] + o[COL_NEG]
        pos += o[COL_POS]
    if pos == 0:
        return (np.float32(0.0), np.float32(0.0))
    return (np.float32(loc / pos), np.float32(conf / pos))
